# revision 1
# baseline (speedup 1.0000x reference)
"""Trainium2 Bass kernel for nn_EuESN_maml: assemble the 3N x 3N wave-equation
transition matrix A (N = 48*48) from c/dkx/dky fields.

The kernel is HBM-write-bound: all 8 NeuronCores share one trn2 device, so the
full f32 output (191 MB) costs ~67 us of device-wide write time alone. The
correctness gate is rel_err < 2e-2, so the device emits the shard in float16
(max rounding 4.9e-4, zeros exact) and the host upcasts while gathering --
halving HBM traffic. Per core the [864, 6912] f16 shard is:

  1. rows 0-576 (sub-bands 0,1): flat contiguous chunk DMAs (sync queue,
     128 descriptors of 1.9-11.7 KB); the first chunk is staged in 3 growing
     pieces chasing a 3-piece zero-tile memset so fill data starts early.
  2. rows 576-864 (sub-band 2): parallelogram zero strips BETWEEN the
     diagonal bands ([[M3+1,cnt],[1,w]]) plus "seam" strips covering each
     row's trailing zeros AND the next row's leading zeros in one descriptor.
     Issued last on the fill queue: nothing depends on them -> no tail.
  3. diagonals: 21 "band" DMAs (split across both HWDGE queues): 256-column
     parallelograms (512 B descriptors) sourced from an SBUF f16 arena whose
     value columns the vector engine writes (with f32->f16 cast) during the
     value computation. Sub-band 0/1 bands overwrite their chunk zeros,
     waiting only for the covering chunks (FIFO completion); sub-band 2
     bands are disjoint from the strips and wait only for the values.

Sharding (SPMD, 8 cores): block-row index partitioned. Core k owns rows
[288k, 288k+288) of each of the three N-row block rows of A -> a [864, 6912]
shard, each 288-row sub-band column-rotated by its first global row index so
diagonal positions are core-invariant (single SPMD program); the host
un-rotates with two slice copies while gathering.
"""

import math
import sys

import numpy as np

sys.path.insert(0, "/opt/trn_rl_repo")

import concourse.bass as bass
import concourse.mybir as mybir
from concourse.bass_utils import run_bass_kernel_spmd

# ---- problem constants (hardcoded from the nn_EuESN_maml spec) ----
n = 48
N = n * n            # 2304
M3 = 3 * N           # 6912 (output is M3 x M3)
NCORES = 8
B = N // NCORES      # 288 rows per sub-band
ROWS = 3 * B         # 864 rows per core shard
DT, CN, KP = 1.0, 0.1, 1e-4
MI = 1.0 / (1.0 / DT - KP / 2.0)          # 1/diagM (diagM is constant)
K0 = (1.0 / DT + KP / 2.0) * MI           # A00 diagonal value (constant)
DXC = (DT / CN) * math.sqrt(2.0)          # dx = DXC * max(c)

W = 256                                   # band window width (512 B f16 descs)
ZW = 5832                                 # zero tile width (108 shard rows)
ZPIECES = [128, 1458, 2916, 5832]         # progressive memset boundaries
PIECES = [(0, 128), (128, 128), (256, 32)]  # 288 rows per sub-band
AW = 21 * W                               # arena: 7 windows x 3 pieces

# windows: (sub_band, base_col, [(delta, slot), ...]); diag col = base + s
WINDOWS = [
    (0, 0,         [(0, "a00")]),
    (0, N - n,     [(0, "a01a"), (n, "a01b")]),
    (0, 2 * N - 1, [(0, "a02a"), (1, "a02b")]),
    (1, 0,         [(0, "a11")]),
    (1, 2 * N,     [(0, "a10a"), (n, "a10b")]),
    (2, 0,         [(0, "a22")]),
    (2, N,         [(0, "a20a"), (1, "a20b")]),
]

# sub-band 2 geometry (rows 576-864 are strip+band tiled, no chunk fill)
S2_STRIPS = [(W, N - W), (N + W, 2240)]   # zero strips between the bands
S2_BK = N + W + 2240                      # seam start col (= 4800)
S2_SEAMW = M3 - S2_BK + 1                 # 2113


def fill_chunks():
    """(flat_offset, n_descs, width, zpiece) contiguous chunks, rows 0-576.

    zpiece = index of the last zero-tile memset piece the chunk's source
    columns [0, width) depend on; widths grow as the memset progresses."""
    out = [(0, 128, 128, 1),                       # rows  0-  2.4
           (128 * 128, 128, 1458, 2),              # rows       - 29.4
           (128 * (128 + 1458), 128, 1330, 2),     # rows       - 54
           (128 * 2916, 128, 2916, 3)]             # rows 54   -108
    for g in range(1, 5):
        out.append((g * 128 * ZW, 128, ZW, 4))     # rows 108-540
    out.append((5 * 128 * ZW, 128, 1944, 4))       # rows 540-576
    return out


def fill_strips():
    """(flat_offset, stride, n_descs, width) zero strips for rows 576-864.

    The final seam descriptor wraps into a padded dummy output row so every
    seam keeps cnt a multiple of 32 (odd counts land whole on one SDMA
    engine, making it the long pole of the contended tail)."""
    r0 = 2 * B
    out = []
    for p0, cnt in PIECES:
        for base, w in S2_STRIPS:
            out.append(((r0 + p0) * M3 + base + p0, M3 + 1, cnt, w))
        out.append(((r0 + p0) * M3 + S2_BK + p0, M3 + 1, cnt, S2_SEAMW))
    return out


def band_descs():
    """(flat_offset, n_descs, window_idx, piece_idx, chunk_wait).
    chunk_wait = fill chunks that must land first (0 for sub-band 2)."""
    chunks = fill_chunks()
    cum = np.cumsum([cnt * w for _, cnt, w, _ in chunks])
    out = []
    for wi, (sub, base, _) in enumerate(WINDOWS):
        r0 = sub * B
        for q, (p0, cnt) in enumerate(PIECES):
            if sub == 2:
                k = 0                              # disjoint from strips
            else:
                k = int(np.searchsorted(cum, (r0 + p0 + cnt) * M3)) + 1
            out.append(((r0 + p0) * M3 + base + p0, cnt, wi, q, k))
    out.sort(key=lambda b: b[4])
    return out


NFILL = len(fill_chunks()) + len(fill_strips())   # 9 + 10
NBAND = len(band_descs())                         # 21


def host_values(c, dkx, dky):
    """Reference value vectors per core/slot (plan_check.py only)."""
    c = np.asarray(c, np.float64)
    dx = DT / CN * c.max() * math.sqrt(2.0)
    cT = c.T.reshape(-1)
    dkxT = np.asarray(dkx, np.float64).T.reshape(-1)
    dkyT = np.asarray(dky, np.float64).T.reshape(-1)
    j = np.arange(N)
    iv = (j // n) / 2.0
    mge = (j >= n).astype(np.float64)
    mmod = (j % n != 0).astype(np.float64)
    mltn = np.where(j < N - n, -1.0, 0.0)
    mmodn = np.where((j + 1) % n != 0, -1.0, 0.0)
    res = []
    for k in range(NCORES):
        sl = slice(k * B, (k + 1) * B)
        rv = cT[sl] / dx
        wv = MI * rv
        gx = dkxT[sl] * iv[sl]
        gy = dkyT[sl] * iv[sl]
        res.append({
            "a00": np.full(B, K0), "a01a": wv * mge[sl], "a01b": -wv,
            "a02a": wv * mmod[sl], "a02b": -wv,
            "a11": (1 - gx) / (1 + gx), "a10a": rv / (1 + gx),
            "a10b": rv / (1 + gx) * mltn[sl],
            "a22": (1 - gy) / (1 + gy), "a20a": rv / (1 + gy),
            "a20b": rv / (1 + gy) * mmodn[sl],
        })
    return res


def build_arena(vals):
    """Numpy mirror of the SBUF f16 arena (plan_check.py only)."""
    arena = np.zeros((128, AW), dtype=np.float16)
    for wi, (_, _, deltas) in enumerate(WINDOWS):
        for q, (p0, cnt) in enumerate(PIECES):
            for delta, slot in deltas:
                arena[0:cnt, (wi * 3 + q) * W + delta] = (
                    vals[slot][p0:p0 + cnt].astype(np.float16))
    return arena


def band_src_cols(wi, q):
    """Columns the band DMA reads, in order (plan_check.py only)."""
    blk = (wi * 3 + q) * W
    return list(range(blk, blk + W))


# ---- packed [128, 3] per-sub-band piece layout: value j -> (j % 128, j//128)
PKCOLS = ["ct", "dkx", "dky", "iv", "mge", "mmod", "mltn", "mmodn"]
PKW = 3 * len(PKCOLS)


def _build_program() -> bass.Bass:
    nc = bass.Bass()
    f32 = mybir.dt.float32
    f16 = mybir.dt.float16

    pk = nc.declare_dram_parameter("pk", [128, PKW], f32, isOutput=False)
    cf = nc.declare_dram_parameter("cf", [1, N], f32, isOutput=False)
    # one dummy row of padding: the last seam descriptor spills into it
    out = nc.declare_dram_parameter("out", [ROWS + 1, M3], f16, isOutput=True)

    chunks = fill_chunks()
    strips = fill_strips()
    bands = band_descs()

    with (
        nc.sbuf_tensor([128, ZW], f16) as zt,          # zero tile
        nc.sbuf_tensor([128, AW], f16) as arena,       # band source windows
        nc.sbuf_tensor([128, PKW], f32) as pkb,        # packed inputs
        nc.sbuf_tensor([1, N], f32) as cfb,            # c (for the max)
        nc.sbuf_tensor([1, 128], f32) as ones,
        nc.sbuf_tensor([1, 128], f32) as bb,           # 1/dx replicated
        nc.sbuf_tensor([128, 1], f32) as invdxp,       # 1/dx per partition
        nc.sbuf_tensor([1, 4], f32) as scal,
        nc.sbuf_tensor([128, 24], f32) as tmp,         # 8 x [128,3] scratch
        nc.semaphore("in_sem") as in_sem,
        nc.semaphore("vchain") as vchain,
        nc.semaphore("bc_sem") as bc_sem,
        nc.semaphore("fill_sem") as fill_sem,
        nc.semaphore("band_sem") as band_sem,
        nc.Block() as block,
    ):
        def pkc(i):                       # [128,3] input block i
            return pkb[0:128, 3 * i : 3 * i + 3]

        IN = {name: pkc(i) for i, name in enumerate(PKCOLS)}

        def t3(i):                        # [128,3] scratch block i
            return tmp[0:128, 3 * i : 3 * i + 3]

        def sc(i):
            return scal[0:1, i : i + 1]

        def aval(wi, delta):              # arena value column, all 3 pieces
            return bass.AP(arena, wi * 3 * W + delta, [[AW, 128], [W, 3]])

        def bsrc(wi, q, cnt):             # contiguous window block
            blk = (wi * 3 + q) * W
            return arena[0:cnt, blk : blk + W]

        mult = mybir.AluOpType.mult
        add = mybir.AluOpType.add

        # vchain indices of the ops other engines wait on
        V_BB, NV = 10, 31

        # the late (k >= 6) bands go on the sync queue after its fill issues,
        # halving the scalar engine's serial band descriptor-gen backlog
        sync_bands = [b for b in bands if b[4] >= 6]
        scalar_bands = [b for b in bands if b[4] < 6]

        @block.sync
        def _(sync):
            for off, cnt, w, zp in chunks:
                sync.wait_ge(vchain, zp)
                sync.dma_start(
                    bass.AP(out, off, [[w, cnt], [1, w]]), zt[0:cnt, 0:w]
                ).then_inc(fill_sem, 16)
            with nc.allow_non_contiguous_dma(reason="strip fill + bands"):
                for off, st, cnt, w in strips:
                    sync.dma_start(
                        bass.AP(out, off, [[st, cnt], [1, w]]), zt[0:cnt, 0:w]
                    ).then_inc(fill_sem, 16)
                sync.wait_ge(vchain, NV)
                for off, cnt, wi, q, k in sync_bands:
                    sync.wait_ge(fill_sem, 16 * k)
                    dst = bass.AP(out, off, [[M3 + 1, cnt], [1, W]])
                    sync.dma_start(dst, bsrc(wi, q, cnt)).then_inc(
                        band_sem, 16)
            sync.wait_ge(fill_sem, 16 * NFILL)

        @block.scalar
        def _(se):
            se.dma_start(pkb[:], pk[:]).then_inc(in_sem, 16)
            se.dma_start(cfb[:], cf[:]).then_inc(in_sem, 16)
            # replicate 1/dx to one value per partition; issued before any
            # bulk traffic sits in the scalar queue (FIFO per queue)
            se.wait_ge(vchain, V_BB)
            se.dma_start(invdxp[0:128, 0:1], bb[0:1, 0:128]).then_inc(
                bc_sem, 16)
            # diagonal bands: values ready at NV; sub-band 0/1 bands wait for
            # the fill chunks covering their rows (FIFO completion); sub-band
            # 2 bands are disjoint from the strips and go immediately
            se.wait_ge(vchain, NV)
            with nc.allow_non_contiguous_dma(reason="diagonal band scatter"):
                for off, cnt, wi, q, k in scalar_bands:
                    if k:
                        se.wait_ge(fill_sem, 16 * k)
                    dst = bass.AP(out, off, [[M3 + 1, cnt], [1, W]])
                    se.dma_start(dst, bsrc(wi, q, cnt)).then_inc(band_sem, 16)
            se.wait_ge(band_sem, 16 * NBAND)

        @block.vector
        def _(v):
            # engines have no scoreboarding: serialize the dependent DVE chain
            # through vchain so each op's writeback lands before the next read
            cnt = [0]

            def step(ins, wait=True):
                cnt[0] += 1
                ins.then_inc(vchain, 1)
                if wait:
                    v.wait_ge(vchain, cnt[0])

            z0 = 0
            for zp in ZPIECES:                                    # 1-4
                # f32 bitcast halves the DVE element count (zeros bit-equal)
                step(v.memset(zt[:, z0:zp].bitcast(f32), 0.0), wait=False)
                z0 = zp
            step(v.memset(arena[:].bitcast(f32), 0.0), wait=False)  # 5
            step(v.memset(ones[:], 1.0), wait=False)              # 6
            v.wait_ge(in_sem, 32)
            # dx = 10*sqrt(2)*max(c); bb = 1/dx replicated along free dim
            step(v.reduce_max(sc(0), cfb[:], axis=mybir.AxisListType.X))  # 6
            step(v.tensor_scalar_mul(sc(1), sc(0), float(DXC)))   # 7
            step(v.reciprocal(sc(2), sc(1)))                      # 8
            step(v.tensor_scalar_mul(bb[:], ones[:], sc(2)))      # 9 = V_BB
            # damping factors: g = dk*iv; A_diag = (1-g)/(1+g)
            step(v.tensor_mul(t3(0), IN["dkx"], IN["iv"]), wait=False)  # 10 gx
            step(v.tensor_mul(t3(1), IN["dky"], IN["iv"]))        # 11 gy
            step(v.tensor_scalar_add(t3(2), t3(0), 1.0), wait=False)    # 11
            step(v.tensor_scalar_add(t3(3), t3(1), 1.0))          # 12
            step(v.tensor_scalar(t3(4), t3(0), -1.0, 1.0, mult, add),
                 wait=False)                                      # 13 1-gx
            step(v.tensor_scalar(t3(5), t3(1), -1.0, 1.0, mult, add))  # 14
            step(v.reciprocal(t3(6), t3(2)), wait=False)          # 15 1/(1+gx)
            step(v.reciprocal(t3(7), t3(3)))                      # 16
            step(v.memset(aval(0, 0), float(K0)), wait=False)     # 17 a00
            step(v.tensor_mul(aval(3, 0), t3(4), t3(6)), wait=False)   # 18 a11
            step(v.tensor_mul(aval(5, 0), t3(5), t3(7)), wait=False)   # 19 a22
            v.wait_ge(bc_sem, 16)
            # rv = c/dx; w = MI*rv; band values (f32 compute, f16 writeback)
            step(v.tensor_scalar_mul(tmp[0:128, 0:3], IN["ct"],
                                     invdxp[0:128, 0:1]))         # 20 rv
            step(v.tensor_scalar_mul(tmp[0:128, 3:6], tmp[0:128, 0:3],
                                     float(MI)))                  # 21 wv
            step(v.tensor_mul(aval(1, 0), tmp[0:128, 3:6], IN["mge"]),
                 wait=False)                                      # 22 a01a
            step(v.tensor_scalar_mul(aval(1, n), tmp[0:128, 3:6], -1.0),
                 wait=False)                                      # 23 a01b
            step(v.tensor_mul(aval(2, 0), tmp[0:128, 3:6], IN["mmod"]),
                 wait=False)                                      # 24 a02a
            step(v.tensor_scalar_mul(aval(2, 1), tmp[0:128, 3:6], -1.0),
                 wait=False)                                      # 25 a02b
            step(v.tensor_mul(aval(4, 0), tmp[0:128, 0:3], t3(6)))  # 26 a10a
            step(v.tensor_mul(aval(4, n), aval(4, 0), IN["mltn"]),
                 wait=False)                                      # 27 a10b
            step(v.tensor_mul(aval(6, 0), tmp[0:128, 0:3], t3(7)))  # 28 a20a
            step(v.tensor_mul(aval(6, 1), aval(6, 0), IN["mmodn"]),
                 wait=False)                                      # 29 a20b
            assert cnt[0] == NV, cnt[0]

    return nc


_nc_cache = None


def _get_nc() -> bass.Bass:
    global _nc_cache
    if _nc_cache is None:
        _nc_cache = _build_program()
    return _nc_cache


def _pack3(v):
    """[288] -> [128, 3]: value j at (j % 128, j // 128), zero-padded."""
    return np.concatenate(
        [np.asarray(v, np.float32), np.zeros(3 * 128 - B, np.float32)]
    ).reshape(3, 128).T.copy()


def _make_in_maps(c, dkx, dky):
    c = np.ascontiguousarray(c, dtype=np.float32)
    cT = np.ascontiguousarray(c.T).reshape(-1)
    dkxT = np.ascontiguousarray(np.asarray(dkx, np.float32).T).reshape(-1)
    dkyT = np.ascontiguousarray(np.asarray(dky, np.float32).T).reshape(-1)
    j = np.arange(N)
    iv = ((j // n) / 2.0).astype(np.float32)
    mge = (j >= n).astype(np.float32)
    mmod = (j % n != 0).astype(np.float32)
    mltn = np.where(j < N - n, -1.0, 0.0).astype(np.float32)
    mmodn = np.where((j + 1) % n != 0, -1.0, 0.0).astype(np.float32)

    cflat = c.reshape(1, N)
    in_maps = []
    for k in range(NCORES):
        sl = slice(k * B, (k + 1) * B)
        blocks = [cT[sl], dkxT[sl], dkyT[sl], iv[sl],
                  mge[sl], mmod[sl], mltn[sl], mmodn[sl]]
        pk = np.concatenate([_pack3(b) for b in blocks], axis=1)
        assert pk.shape == (128, PKW)
        in_maps.append({"pk": pk, "cf": cflat})
    return in_maps


def _assemble(shards) -> np.ndarray:
    A = np.zeros((M3, M3), dtype=np.float32)
    for k in range(NCORES):
        shard = shards[k]
        for b in range(3):
            g0 = b * N + k * B
            band = shard[b * B : (b + 1) * B].astype(np.float32)
            if g0:
                A[g0 : g0 + B, g0:] = band[:, : M3 - g0]
                A[g0 : g0 + B, :g0] = band[:, M3 - g0 :]
            else:
                A[:B, :] = band
    return A


def kernel(c, dkx, dky, _trace=False):
    in_maps = _make_in_maps(c, dkx, dky)
    res = run_bass_kernel_spmd(
        _get_nc(), in_maps, core_ids=list(range(NCORES)), trace=_trace
    )
    A = _assemble([res.results[k]["out"][:ROWS] for k in range(NCORES)])
    if _trace:
        return A, res
    return A

